# revision 54
# baseline (speedup 1.0000x reference)
"""Distributed Trainium2 Bass kernel for an attention block (fused, v3).

Reference math (B=2, S=2048, H=2048, NH=16, HD=128):
  qkv = x @ Wqkv.T -> split q,k,v per head -> RoPE(q,k via frequency_cis 2x2)
  scores = (q @ k.T) * 1/sqrt(HD) + causal mask -> softmax -> @ v -> @ Wout.T

Sharding (8 cores): core c handles batch b=c//4 and heads 4*(c%4)..4*(c%4)+3.

Structure (single launch, fully pipelined over s-tiles of 512):
  for n in 0..3:
    proj(n):      q/k (RoPE'd, [HD, s] layout) and v ([s, hd] natural) for
                  s in [512n, 512n+512)
    attention(n): transposed-score flash attention for q-tile n against key
                  chunks 0..4n+3 (causal skip); probsT = exp(scT) feeds PV
                  directly (keys on partitions) -- zero transposes.
                  l[q] via a ones-row matmul accumulated beside PV.
    AllGather(n): this q-slab of attnT across the 4 same-batch cores
                  (slab-major internal DRAM so each AG is contiguous),
                  overlapped with the next iteration's compute.
    out-proj(n-1): Wout col-split matmul for the previous slab.
"""

import numpy as np
import ml_dtypes
from contextlib import ExitStack

B, S, H, NH, HD = 2, 2048, 2048, 16, 128
NHL = 4          # heads per core
NCORES = 8
SCALE = 1.0 / np.sqrt(HD)
BF16 = ml_dtypes.bfloat16
NEG = -1e9

_cache = {}


def _build():
    import concourse.bass as bass
    import concourse.tile as tile
    from concourse import bacc, mybir
    dt = mybir.dt
    nc = bacc.Bacc("TRN2", target_bir_lowering=False, debug=False,
                   num_devices=NCORES)

    P = 128
    KO = H // P           # 16 contraction chunks for the projections
    NQT = S // 512        # 4 q tiles of 512

    xT = nc.dram_tensor("xT", [H, S], dt.bfloat16, kind="ExternalInput").ap()
    # block-major: wqkT[i] = columns [128i, 128i+128) of Wqk.T, contiguous
    # so the per-matmul-group prologue loads are single fast DMAs
    wqkT = nc.dram_tensor("wqkT", [2 * NHL, H, HD], dt.bfloat16,
                          kind="ExternalInput").ap()
    wvT = nc.dram_tensor("wvT", [H, NHL * HD], dt.bfloat16,
                         kind="ExternalInput").ap()
    rope = nc.dram_tensor("rope", [2, HD, S], dt.float32,
                          kind="ExternalInput").ap()
    mtri = nc.dram_tensor("mtri", [P, P], dt.float32,
                          kind="ExternalInput").ap()
    woutT = nc.dram_tensor("woutT", [H, 512], dt.bfloat16,
                           kind="ExternalInput").ap()
    out_ext = nc.dram_tensor("out", [S, 512], dt.float32,
                             kind="ExternalOutput").ap()

    # slab-major internal DRAM for the AllGathers; each slab gathers in two
    # half-head chunks (heads {0,1} then {2,3}) so the second half of the
    # last slab's AG is all that remains exposed at the end. Each AG takes
    # ~15-35us wall here, so out_proj consumes two iterations later.
    atl = nc.dram_tensor("attnT_loc", [NQT, NHL * P, 512], dt.bfloat16)
    ats = nc.dram_tensor("attnT_sh", [NQT, 2, 4 * 2 * P, 512], dt.bfloat16)

    with tile.TileContext(nc) as tc, ExitStack() as ctx:
        per = ctx.enter_context(tc.tile_pool(name="per", bufs=1))
        sb = ctx.enter_context(tc.tile_pool(name="sb", bufs=1))
        ps = ctx.enter_context(tc.tile_pool(name="ps", bufs=1, space="PSUM"))

        # persistent: roped q/k in [HD, h, S]; v natural [s%P, s//P, h*HD+d]
        qsb = per.tile([P, NHL, S], dt.bfloat16, tag="qsb")
        ksb = per.tile([P, NHL, S], dt.bfloat16, tag="ksb")
        vsb = per.tile([P, KO, NHL * HD], dt.bfloat16, tag="vsb")
        ones = per.tile([P, 1], dt.bfloat16, tag="ones")
        nc.vector.memset(ones[:], 1.0)

        # ---- prologue loads: ordered by first consumption so the first
        # matmul group is fed after ~0.75MB instead of ~10MB ----
        xTr = xT.rearrange("(ko p) s -> p ko s", p=P)
        wqkr = wqkT.rearrange("b (ko p) m -> b p ko m", p=P)
        wqk = per.tile([P, 2 * NHL, KO, HD], dt.bfloat16, tag="wqk")
        xn0 = sb.tile([P, KO, 512], dt.bfloat16, tag="xn", bufs=1)
        rsb0 = sb.tile([P, 2, 512], dt.float32, tag="rsb", bufs=2)
        wv = per.tile([P, KO, NHL * HD], dt.bfloat16, tag="wv")
        msb = per.tile([P, P], dt.float32, tag="msb")
        nc.sync.dma_start(msb[:], mtri)   # tiny; warms the DMA path
        xchunks = [(0, 2), (2, 4), (4, 8), (8, 12), (12, 16)]
        for i in range(8):
            nc.sync.dma_start(wqk[:, i], wqkr[i])
            if i < len(xchunks):
                ck = slice(*xchunks[i])
                nc.sync.dma_start(xn0[:, ck, :], xTr[:, ck, 0:512])
            # big loads slotted by first-consumption time so they don't
            # delay later weight blocks behind them in the DMA queue
            if i == 1:
                nc.sync.dma_start(
                    rsb0[:], rope.rearrange("r p s -> p r s")[:, :, 0:512])
            if i == 5:
                nc.sync.dma_start(wv[:],
                                  wvT.rearrange("(ko p) m -> p ko m", p=P))
        wo = per.tile([P, KO, 512], dt.bfloat16, tag="wo")
        nc.sync.dma_start(wo[:], woutT.rearrange("(ko p) n -> p ko n", p=P))


        def proj(n, xn, rsb):
            ns = slice(n * 512, (n + 1) * 512)
            for h in range(NHL):
                for t in range(2):   # q, k with RoPE
                    pp = ps.tile([P, 512], dt.float32, tag="pmm", bufs=2,
                                 name="pp")
                    for kc in range(KO):
                        nc.tensor.matmul(
                            pp[:], wqk[:, h * 2 + t, kc, :], xn[:, kc, :],
                            start=(kc == 0), stop=(kc == KO - 1))
                    # rope input holds [A, swap(B)]; u = q*swap(B), then
                    # DMA-swap u's partition halves so t2 = swap(q)*B,
                    # and dst = q*A + t2.
                    dst = qsb if t == 0 else ksb
                    t1 = sb.tile([P, 512], dt.float32, tag="t1", bufs=2)
                    u = sb.tile([P, 512], dt.float32, tag="u", bufs=2)
                    t2 = sb.tile([P, 512], dt.float32, tag="t2", bufs=2)
                    nc.vector.tensor_tensor(t1[:], pp[:], rsb[:, 0, :],
                                            mybir.AluOpType.mult)
                    nc.vector.tensor_tensor(u[:], pp[:], rsb[:, 1, :],
                                            mybir.AluOpType.mult)
                    # swaps issued from Scalar (idle during proj): gpsimd
                    # blocks on collectives, and sync's at/out-DMAs would
                    # head-of-line-block these behind the finish chain
                    nc.scalar.dma_start(t2[:64], u[64:, :])
                    nc.scalar.dma_start(t2[64:], u[:64, :])
                    nc.vector.tensor_tensor(dst[:, h, ns], t1[:], t2[:],
                                            mybir.AluOpType.add)
            # v in natural layout: stationary = x s-cols, moving = Wv.
            # evacuation on Scalar (idle during proj) keeps the DVE FIFO
            # clear for the rope chain.
            for j in range(4):
                pv = ps.tile([P, 512], dt.float32, tag="pmm", bufs=2,
                             name="pv")
                for kc in range(KO):
                    nc.tensor.matmul(
                        pv[:], xn[:, kc, j * P:(j + 1) * P], wv[:, kc, :],
                        start=(kc == 0), stop=(kc == KO - 1))
                nc.scalar.copy(vsb[:, n * 4 + j, :], pv[:])

        asb_tiles = {}   # (qt, c) -> sbuf tile [P, 8, 512]

        def post_chunk(qt, c):
            """AllGather half a q-slab (heads 2c,2c+1) and prefetch it."""
            nc.gpsimd.collective_compute(
                "AllGather",
                mybir.AluOpType.bypass,
                replica_groups=[[0, 1, 2, 3], [4, 5, 6, 7]],
                ins=[atl.ap()[qt, c * 2 * P:(c + 1) * 2 * P, :].opt()],
                outs=[ats.ap()[qt, c].opt()],
            )
            # prefetch on gpsimd: the load waits on the AG semaphore, and a
            # sync-queue wait would head-of-line-block the rope swaps and
            # x loads queued behind it
            asb = sb.tile([P, 8, 512], dt.bfloat16, tag="asb", bufs=4,
                          name="asbc")
            nc.gpsimd.dma_start(
                asb[:], ats.ap()[qt, c].rearrange("(k p) s -> p k s", p=P))
            asb_tiles[(qt, c)] = asb

        def attention(qt):
            qs = slice(qt * 512, (qt + 1) * 512)
            emitted = []
            # per-head unnormalized pv (SBUF) + the 4 heads' l rows; the
            # normalization runs once per slab so the Scalar engine does a
            # single Ln/Exp table swap per slab instead of one per head.
            lall = sb.tile([1, NHL, 512], dt.float32, tag="lall", bufs=1,
                           name="lall")
            pv_sbs = []

            def flush_one():
                kc, et, st, q0 = emitted.pop(0)
                first, last = kc == 0, kc == st["nkc"] - 1
                nc.tensor.matmul(st["l"][0:1, q0:], ones[:], et[:, q0:],
                                 start=first, stop=last)
                nc.tensor.matmul(st["pv"][:, q0:], vsb[:, kc, st["h"] * P:
                                                      (st["h"] + 1) * P],
                                 et[:, q0:], start=first, stop=last)
                if last:
                    h = st["h"]
                    nc.scalar.copy(lall[0:1, h, :], st["l"][0:1, :])
                    pvs = sb.tile([P, 512], dt.bfloat16, tag="pvs", bufs=4,
                                  name="pvs")
                    nc.vector.tensor_copy(pvs[:], st["pv"][:])
                    pv_sbs.append(pvs)

            for c in range(2):
                for h in (2 * c, 2 * c + 1):
                    nkc = 4 * qt + 4
                    st = {
                        "h": h, "nkc": nkc,
                        "pv": ps.tile([P, 512], dt.float32, tag="ppv",
                                      bufs=2, name="pvacc"),
                        "l": ps.tile([P, 512], dt.float32, tag="pl", bufs=1,
                                     name="lacc"),
                    }
                    for kc in range(nkc):
                        # columns < q0 of this chunk are fully causally
                        # masked: restrict every op to the active [q0:]
                        # range (the skipped region contributes exact
                        # zeros to l and pv)
                        j = kc - 4 * qt
                        q0 = j * P if j > 0 else 0
                        sc = ps.tile([P, 512], dt.float32, tag="psc",
                                     bufs=3, name="sc")
                        nc.tensor.matmul(sc[:, q0:],
                                         ksb[:, h, kc * P:(kc + 1) * P],
                                         qsb[:, h, qt * 512 + q0:
                                             (qt + 1) * 512],
                                         start=True, stop=True)
                        et = sb.tile([P, 512], dt.bfloat16, tag="et",
                                     bufs=4)
                        if j >= 0:   # straddles the causal diagonal
                            nc.vector.tensor_tensor(
                                sc[:, j * P:(j + 1) * P],
                                sc[:, j * P:(j + 1) * P],
                                msb[:], mybir.AluOpType.add)
                        nc.scalar.activation(
                            et[:, q0:], sc[:, q0:],
                            mybir.ActivationFunctionType.Exp)
                        emitted.append((kc, et, st, q0))
                        while len(emitted) > 2:
                            flush_one()
                while emitted:
                    flush_one()

                # half-slab normalization: rl = exp(-ln(l)) for 2 heads,
                # then kick this chunk's AllGather immediately
                nc.scalar.activation(lall[0:1, 2 * c:2 * c + 2, :],
                                     lall[0:1, 2 * c:2 * c + 2, :],
                                     mybir.ActivationFunctionType.Ln)
                nc.scalar.activation(lall[0:1, 2 * c:2 * c + 2, :],
                                     lall[0:1, 2 * c:2 * c + 2, :],
                                     mybir.ActivationFunctionType.Exp,
                                     scale=-1.0)
                for h in (2 * c, 2 * c + 1):
                    lb = sb.tile([P, 512], dt.float32, tag="lb", bufs=2)
                    nc.gpsimd.partition_broadcast(lb[:], lall[0:1, h, :])
                    at = sb.tile([P, 512], dt.bfloat16, tag="at", bufs=2)
                    nc.vector.tensor_tensor(at[:], pv_sbs[h][:], lb[:],
                                            mybir.AluOpType.mult)
                    nc.sync.dma_start(atl.ap()[qt, h * P:(h + 1) * P, :],
                                      at[:])
                post_chunk(qt, c)

        def out_proj(qt):
            asbs = [asb_tiles.pop((qt, c)) for c in range(2)]
            for mq in range(4):
                po = ps.tile([P, 512], dt.float32, tag="pmm", bufs=2,
                             name="po")
                i = 0
                for c in range(2):
                    for g in range(4):
                        for hh in range(2):
                            # gathered chunk (c, g, hh) = global head
                            # g*4 + 2c + hh
                            nc.tensor.matmul(
                                po[:],
                                asbs[c][:, g * 2 + hh,
                                        mq * P:(mq + 1) * P],
                                wo[:, g * 4 + 2 * c + hh, :],
                                start=(i == 0), stop=(i == KO - 1))
                            i += 1
                ev = sb.tile([P, 512], dt.float32, tag="ev", bufs=2)
                nc.vector.tensor_copy(ev[:], po[:])
                nc.sync.dma_start(
                    out_ext[(qt * 4 + mq) * P:(qt * 4 + mq + 1) * P, :],
                    ev[:])

        rr = rope.rearrange("r p s -> p r s")
        xn_t, rsb_t = xn0, rsb0
        for n in range(NQT):
            proj(n, xn_t, rsb_t)
            if n + 1 < NQT:
                ns2 = slice((n + 1) * 512, (n + 2) * 512)
                xn_t = sb.tile([P, KO, 512], dt.bfloat16, tag="xn", bufs=1,
                               name="xn_n")
                nc.sync.dma_start(xn_t[:], xTr[:, :, ns2])
                rsb_t = sb.tile([P, 2, 512], dt.float32, tag="rsb", bufs=2,
                                name="rsb_n")
                nc.sync.dma_start(rsb_t[:], rr[:, :, ns2])
            attention(n)
            if n >= 2:
                out_proj(n - 2)
        out_proj(NQT - 2)
        out_proj(NQT - 1)

    nc.compile()
    return nc


def _host_prep(x, attention_mask, frequency_cis, Wqkv, Wout):
    """Build the 8 per-core input maps (numpy only)."""
    x = np.asarray(x, dtype=np.float32)
    fc = np.asarray(frequency_cis, dtype=np.float32)
    Wqkv = np.asarray(Wqkv, dtype=np.float32)
    Wout = np.asarray(Wout, dtype=np.float32)

    # rotate-half permutation of the head dim: new row p<64 <- old 2p,
    # p>=64 <- old 2(p-64)+1
    perm = np.concatenate([np.arange(0, HD, 2), np.arange(1, HD, 2)])
    # rope coefficients in permuted layout: [A;B] each [HD, S]
    ropeA = np.concatenate([fc[:, :, 0, 0].T, fc[:, :, 1, 1].T], axis=0)
    ropeBsw = np.concatenate([fc[:, :, 1, 0].T, fc[:, :, 0, 1].T], axis=0)
    rope = np.stack([ropeA, ropeBsw]).astype(np.float32)  # [2, HD, S]

    # strict upper triangle masked: key i > query c
    mtri = np.where(np.arange(128)[:, None] > np.arange(128)[None, :],
                    np.float32(NEG), np.float32(0.0)).astype(np.float32)

    xT = [np.ascontiguousarray(x[b].T).astype(BF16) for b in range(B)]
    woutT_f = Wout.T.astype(np.float32)                  # [H(in), H(out)]
    wout_slices = [np.ascontiguousarray(
        woutT_f[:, g * 512:(g + 1) * 512]).astype(BF16) for g in range(4)]

    in_maps = []
    for c in range(NCORES):
        b, g = divmod(c, 4)
        qk_rows = []
        v_rows = []
        for j in range(NHL):
            hh = (g * NHL + j) * HD
            qk_rows.append(Wqkv[0 * H + hh:0 * H + hh + HD][perm] * SCALE)
            qk_rows.append(Wqkv[1 * H + hh:1 * H + hh + HD][perm])
            v_rows.append(Wqkv[2 * H + hh:2 * H + hh + HD])
        # block-major: [8 blocks, H, 128], block i = rows of (head i//2,
        # q if i%2==0 else k)
        wqk = np.stack([r.T for r in qk_rows])           # [8, H, 128]
        wv = np.concatenate(v_rows, axis=0)              # [512, H]
        in_maps.append({
            "xT": xT[b],
            "wqkT": np.ascontiguousarray(wqk).astype(BF16),
            "wvT": np.ascontiguousarray(wv.T).astype(BF16),
            "rope": rope,
            "mtri": mtri,
            "woutT": wout_slices[g],
        })
    return in_maps


def _install_ntff_hook():
    """The image's antenv lacks axon_hooks; shim it so trace=True works."""
    import sys
    import types
    import ctypes
    import contextlib
    if "antenv.axon_hooks" in sys.modules:
        return
    mod = types.ModuleType("antenv.axon_hooks")
    _reg = {"hook": None}
    mod.set_axon_ntff_profile_hook = lambda h: _reg.__setitem__("hook", h)
    mod.get_axon_ntff_profile_hook = lambda: _reg["hook"]
    sys.modules["antenv.axon_hooks"] = mod

    so_path = "/opt/axon/libaxon_pjrt.so"
    try:
        lib = ctypes.CDLL(so_path)
        if not hasattr(lib, "axon_start_nrt_profile"):
            return
        lib.axon_start_nrt_profile.argtypes = [
            ctypes.POINTER(ctypes.c_int64), ctypes.c_size_t]
        lib.axon_start_nrt_profile.restype = ctypes.c_int64
        lib.axon_stop_nrt_profile.argtypes = [ctypes.c_char_p]
        lib.axon_stop_nrt_profile.restype = ctypes.c_int64

        @contextlib.contextmanager
        def _hook(output_dir, device_ids):
            import jax
            jax.devices()
            if device_ids:
                ids = (ctypes.c_int64 * len(device_ids))(*device_ids)
                rc = lib.axon_start_nrt_profile(ids, len(device_ids))
            else:
                rc = lib.axon_start_nrt_profile(None, 0)
            if rc != 0:
                raise RuntimeError(f"axon_start_nrt_profile rc={rc}")
            try:
                yield
            finally:
                n = lib.axon_stop_nrt_profile(str(output_dir).encode())
                print(f"profile: {n} file(s) written to {output_dir}")

        mod.set_axon_ntff_profile_hook(_hook)
    except OSError:
        pass


def _run(in_maps, trace=False):
    if trace:
        _install_ntff_hook()
    from concourse.bass_utils import run_bass_kernel_spmd
    if "nc" not in _cache:
        _cache["nc"] = _build()
    return run_bass_kernel_spmd(_cache["nc"], in_maps,
                                list(range(NCORES)), trace=trace)


def _assemble(r):
    out = np.empty((B, S, H), dtype=np.float32)
    for c in range(NCORES):
        b, g = divmod(c, 4)
        out[b, :, g * 512:(g + 1) * 512] = r.results[c]["out"]
    return out


def kernel(x, attention_mask, frequency_cis, Wqkv, Wout):
    in_maps = _host_prep(x, attention_mask, frequency_cis, Wqkv, Wout)
    r = _run(in_maps)
    return _assemble(r)


def kernel_traced(x, attention_mask, frequency_cis, Wqkv, Wout):
    """Like kernel() but also returns (out, exec_time_ns)."""
    in_maps = _host_prep(x, attention_mask, frequency_cis, Wqkv, Wout)
    r = _run(in_maps, trace=True)
    return _assemble(r), getattr(r, "exec_time_ns", None)


# revision 56
# speedup vs baseline: 1.0098x; 1.0098x over previous
"""Distributed Trainium2 Bass kernel for an attention block (fused, v3).

Reference math (B=2, S=2048, H=2048, NH=16, HD=128):
  qkv = x @ Wqkv.T -> split q,k,v per head -> RoPE(q,k via frequency_cis 2x2)
  scores = (q @ k.T) * 1/sqrt(HD) + causal mask -> softmax -> @ v -> @ Wout.T

Sharding (8 cores): core c handles batch b=c//4 and heads 4*(c%4)..4*(c%4)+3.

Structure (single launch, fully pipelined over s-tiles of 512):
  for n in 0..3:
    proj(n):      q/k (RoPE'd, [HD, s] layout) and v ([s, hd] natural) for
                  s in [512n, 512n+512)
    attention(n): transposed-score flash attention for q-tile n against key
                  chunks 0..4n+3 (causal skip); probsT = exp(scT) feeds PV
                  directly (keys on partitions) -- zero transposes.
                  l[q] via a ones-row matmul accumulated beside PV.
    AllGather(n): this q-slab of attnT across the 4 same-batch cores
                  (slab-major internal DRAM so each AG is contiguous),
                  overlapped with the next iteration's compute.
    out-proj(n-1): Wout col-split matmul for the previous slab.
"""

import numpy as np
import ml_dtypes
from contextlib import ExitStack

B, S, H, NH, HD = 2, 2048, 2048, 16, 128
NHL = 4          # heads per core
NCORES = 8
SCALE = 1.0 / np.sqrt(HD)
BF16 = ml_dtypes.bfloat16
NEG = -1e9

_cache = {}


def _build():
    import concourse.bass as bass
    import concourse.tile as tile
    from concourse import bacc, mybir
    dt = mybir.dt
    nc = bacc.Bacc("TRN2", target_bir_lowering=False, debug=False,
                   num_devices=NCORES)

    P = 128
    KO = H // P           # 16 contraction chunks for the projections
    NQT = S // 512        # 4 q tiles of 512

    xT = nc.dram_tensor("xT", [H, S], dt.bfloat16, kind="ExternalInput").ap()
    # block-major: wqkT[i] = columns [128i, 128i+128) of Wqk.T, contiguous
    # so the per-matmul-group prologue loads are single fast DMAs
    wqkT = nc.dram_tensor("wqkT", [2 * NHL, H, HD], dt.bfloat16,
                          kind="ExternalInput").ap()
    wvT = nc.dram_tensor("wvT", [H, NHL * HD], dt.bfloat16,
                         kind="ExternalInput").ap()
    rope = nc.dram_tensor("rope", [2, HD, S], dt.float32,
                          kind="ExternalInput").ap()
    mtri = nc.dram_tensor("mtri", [P, P], dt.float32,
                          kind="ExternalInput").ap()
    woutT = nc.dram_tensor("woutT", [H, 512], dt.bfloat16,
                           kind="ExternalInput").ap()
    out_ext = nc.dram_tensor("out", [S, 512], dt.float32,
                             kind="ExternalOutput").ap()

    # slab-major internal DRAM for the AllGathers; each slab gathers in two
    # half-head chunks (heads {0,1} then {2,3}) so the second half of the
    # last slab's AG is all that remains exposed at the end. Each AG takes
    # ~15-35us wall here, so out_proj consumes two iterations later.
    atl = nc.dram_tensor("attnT_loc", [NQT, NHL * P, 512], dt.bfloat16)
    ats = nc.dram_tensor("attnT_sh", [NQT, 2, 4 * 2 * P, 512], dt.bfloat16)

    with tile.TileContext(nc) as tc, ExitStack() as ctx:
        per = ctx.enter_context(tc.tile_pool(name="per", bufs=1))
        sb = ctx.enter_context(tc.tile_pool(name="sb", bufs=1))
        ps = ctx.enter_context(tc.tile_pool(name="ps", bufs=1, space="PSUM"))

        # persistent: roped q/k in [HD, h, S]; v natural [s%P, s//P, h*HD+d]
        qsb = per.tile([P, NHL, S], dt.bfloat16, tag="qsb")
        ksb = per.tile([P, NHL, S], dt.bfloat16, tag="ksb")
        vsb = per.tile([P, KO, NHL * HD], dt.bfloat16, tag="vsb")
        ones = per.tile([P, 1], dt.bfloat16, tag="ones")
        nc.vector.memset(ones[:], 1.0)

        # ---- prologue loads: ordered by first consumption so the first
        # matmul group is fed after ~0.75MB instead of ~10MB ----
        xTr = xT.rearrange("(ko p) s -> p ko s", p=P)
        wqkr = wqkT.rearrange("b (ko p) m -> b p ko m", p=P)
        wqk = per.tile([P, 2 * NHL, KO, HD], dt.bfloat16, tag="wqk")
        xn0 = sb.tile([P, KO, 512], dt.bfloat16, tag="xn", bufs=1)
        rsb0 = sb.tile([P, 2, 512], dt.float32, tag="rsb", bufs=2)
        wv = per.tile([P, KO, NHL * HD], dt.bfloat16, tag="wv")
        msb = per.tile([P, P], dt.float32, tag="msb")
        nc.sync.dma_start(msb[:], mtri)   # tiny; warms the DMA path
        xchunks = [(0, 2), (2, 4), (4, 8), (8, 12), (12, 16)]
        for i in range(8):
            nc.sync.dma_start(wqk[:, i], wqkr[i])
            if i < len(xchunks):
                ck = slice(*xchunks[i])
                nc.sync.dma_start(xn0[:, ck, :], xTr[:, ck, 0:512])
            # big loads slotted by first-consumption time so they don't
            # delay later weight blocks behind them in the DMA queue
            if i == 1:
                nc.sync.dma_start(
                    rsb0[:], rope.rearrange("r p s -> p r s")[:, :, 0:512])
            if i == 5:
                nc.sync.dma_start(wv[:],
                                  wvT.rearrange("(ko p) m -> p ko m", p=P))
        wo = per.tile([P, KO, 512], dt.bfloat16, tag="wo")
        nc.sync.dma_start(wo[:], woutT.rearrange("(ko p) n -> p ko n", p=P))


        def proj(n, xn, rsb):
            ns = slice(n * 512, (n + 1) * 512)
            for h in range(NHL):
                for t in range(2):   # q, k with RoPE
                    pp = ps.tile([P, 512], dt.float32, tag="pmm", bufs=2,
                                 name="pp")
                    for kc in range(KO):
                        nc.tensor.matmul(
                            pp[:], wqk[:, h * 2 + t, kc, :], xn[:, kc, :],
                            start=(kc == 0), stop=(kc == KO - 1))
                    # rope input holds [A, swap(B)]; u = q*swap(B), then
                    # DMA-swap u's partition halves so t2 = swap(q)*B,
                    # and dst = q*A + t2.
                    dst = qsb if t == 0 else ksb
                    t1 = sb.tile([P, 512], dt.float32, tag="t1", bufs=2)
                    u = sb.tile([P, 512], dt.float32, tag="u", bufs=2)
                    t2 = sb.tile([P, 512], dt.float32, tag="t2", bufs=2)
                    nc.vector.tensor_tensor(t1[:], pp[:], rsb[:, 0, :],
                                            mybir.AluOpType.mult)
                    nc.vector.tensor_tensor(u[:], pp[:], rsb[:, 1, :],
                                            mybir.AluOpType.mult)
                    # swaps issued from Scalar (idle during proj): gpsimd
                    # blocks on collectives, and sync's at/out-DMAs would
                    # head-of-line-block these behind the finish chain
                    nc.scalar.dma_start(t2[:64], u[64:, :])
                    nc.scalar.dma_start(t2[64:], u[:64, :])
                    nc.vector.tensor_tensor(dst[:, h, ns], t1[:], t2[:],
                                            mybir.AluOpType.add)
            # v in natural layout: stationary = x s-cols, moving = Wv.
            # evacuation on Scalar (idle during proj) keeps the DVE FIFO
            # clear for the rope chain.
            for j in range(4):
                pv = ps.tile([P, 512], dt.float32, tag="pmm", bufs=2,
                             name="pv")
                for kc in range(KO):
                    nc.tensor.matmul(
                        pv[:], xn[:, kc, j * P:(j + 1) * P], wv[:, kc, :],
                        start=(kc == 0), stop=(kc == KO - 1))
                nc.scalar.copy(vsb[:, n * 4 + j, :], pv[:])

        asb_tiles = {}   # (qt, c) -> sbuf tile [P, 8, 512]

        def post_chunk(qt, c):
            """AllGather half a q-slab (heads 2c,2c+1) and prefetch it."""
            nc.gpsimd.collective_compute(
                "AllGather",
                mybir.AluOpType.bypass,
                replica_groups=[[0, 1, 2, 3], [4, 5, 6, 7]],
                ins=[atl.ap()[qt, c * 2 * P:(c + 1) * 2 * P, :].opt()],
                outs=[ats.ap()[qt, c].opt()],
            )
            # prefetch on gpsimd: the load waits on the AG semaphore, and a
            # sync-queue wait would head-of-line-block the rope swaps and
            # x loads queued behind it
            asb = sb.tile([P, 8, 512], dt.bfloat16, tag="asb", bufs=4,
                          name="asbc")
            nc.gpsimd.dma_start(
                asb[:], ats.ap()[qt, c].rearrange("(k p) s -> p k s", p=P))
            asb_tiles[(qt, c)] = asb

        def attention(qt):
            qs = slice(qt * 512, (qt + 1) * 512)
            emitted = []
            # per-head unnormalized pv (SBUF) + the 4 heads' l rows; the
            # normalization runs once per slab so the Scalar engine does a
            # single Ln/Exp table swap per slab instead of one per head.
            lall = sb.tile([1, NHL, 512], dt.float32, tag="lall", bufs=1,
                           name="lall")
            pv_sbs = []

            def flush_one():
                kc, et, st, q0 = emitted.pop(0)
                first, last = kc == 0, kc == st["nkc"] - 1
                nc.tensor.matmul(st["l"][0:1, q0:], ones[:], et[:, q0:],
                                 start=first, stop=last)
                nc.tensor.matmul(st["pv"][:, q0:], vsb[:, kc, st["h"] * P:
                                                      (st["h"] + 1) * P],
                                 et[:, q0:], start=first, stop=last)
                if last:
                    h = st["h"]
                    nc.scalar.copy(lall[0:1, h, :], st["l"][0:1, :])
                    pvs = sb.tile([P, 512], dt.bfloat16, tag="pvs", bufs=4,
                                  name="pvs")
                    nc.vector.tensor_copy(pvs[:], st["pv"][:])
                    pv_sbs.append(pvs)

            for c in range(2):
                for h in (2 * c, 2 * c + 1):
                    nkc = 4 * qt + 4
                    st = {
                        "h": h, "nkc": nkc,
                        "pv": ps.tile([P, 512], dt.float32, tag="ppv",
                                      bufs=2, name="pvacc"),
                        "l": ps.tile([P, 512], dt.float32, tag="pl", bufs=1,
                                     name="lacc"),
                    }
                    for kc in range(nkc):
                        # columns < q0 of this chunk are fully causally
                        # masked: restrict every op to the active [q0:]
                        # range (the skipped region contributes exact
                        # zeros to l and pv)
                        j = kc - 4 * qt
                        q0 = j * P if j > 0 else 0
                        sc = ps.tile([P, 512], dt.float32, tag="psc",
                                     bufs=3, name="sc")
                        nc.tensor.matmul(sc[:, q0:],
                                         ksb[:, h, kc * P:(kc + 1) * P],
                                         qsb[:, h, qt * 512 + q0:
                                             (qt + 1) * 512],
                                         start=True, stop=True)
                        et = sb.tile([P, 512], dt.bfloat16, tag="et",
                                     bufs=4)
                        if j >= 0:   # straddles the causal diagonal
                            nc.vector.tensor_tensor(
                                sc[:, j * P:(j + 1) * P],
                                sc[:, j * P:(j + 1) * P],
                                msb[:], mybir.AluOpType.add)
                        nc.scalar.activation(
                            et[:, q0:], sc[:, q0:],
                            mybir.ActivationFunctionType.Exp)
                        emitted.append((kc, et, st, q0))
                        while len(emitted) > 2:
                            flush_one()
                while emitted:
                    flush_one()

                # half-slab normalization: rl = exp(-ln(l)) for 2 heads,
                # then kick this chunk's AllGather immediately
                nc.scalar.activation(lall[0:1, 2 * c:2 * c + 2, :],
                                     lall[0:1, 2 * c:2 * c + 2, :],
                                     mybir.ActivationFunctionType.Ln)
                nc.scalar.activation(lall[0:1, 2 * c:2 * c + 2, :],
                                     lall[0:1, 2 * c:2 * c + 2, :],
                                     mybir.ActivationFunctionType.Exp,
                                     scale=-1.0)
                for h in (2 * c, 2 * c + 1):
                    lb = sb.tile([P, 512], dt.float32, tag="lb", bufs=2)
                    nc.gpsimd.partition_broadcast(lb[:], lall[0:1, h, :])
                    at = sb.tile([P, 512], dt.bfloat16, tag="at", bufs=2)
                    nc.vector.tensor_tensor(at[:], pv_sbs[h][:], lb[:],
                                            mybir.AluOpType.mult)
                    nc.sync.dma_start(atl.ap()[qt, h * P:(h + 1) * P, :],
                                      at[:])
                post_chunk(qt, c)

        def out_proj(qt, split=False):
            asbs = [asb_tiles.pop((qt, c)) for c in range(2)]

            def chunk_mms(po, mq, c, first, last):
                for i in range(8):
                    g, hh = divmod(i, 2)
                    # gathered chunk (c, g, hh) = global head g*4 + 2c + hh
                    nc.tensor.matmul(
                        po[:],
                        asbs[c][:, g * 2 + hh, mq * P:(mq + 1) * P],
                        wo[:, g * 4 + 2 * c + hh, :],
                        start=(first and i == 0), stop=(last and i == 7))

            def evac(po, mq):
                ev = sb.tile([P, 512], dt.float32, tag="ev", bufs=2)
                nc.vector.tensor_copy(ev[:], po[:])
                nc.sync.dma_start(
                    out_ext[(qt * 4 + mq) * P:(qt * 4 + mq + 1) * P, :],
                    ev[:])

            if not split:
                for mq in range(4):
                    po = ps.tile([P, 512], dt.float32, tag="pmm", bufs=2,
                                 name="po")
                    chunk_mms(po, mq, 0, True, False)
                    chunk_mms(po, mq, 1, False, True)
                    evac(po, mq)
            else:
                # final slab: run every chunk-A matmul first (they only
                # need the earlier AllGather) so the PE keeps working
                # while chunk B's AG completes; borrows the now-idle
                # attention psum tags for the two extra accumulators
                pos = []
                for mq in range(4):
                    tag, bufs = (("pmm", 2) if mq < 2 else ("psc", 3))
                    po = ps.tile([P, 512], dt.float32, tag=tag, bufs=bufs,
                                 name="pof")
                    pos.append(po)
                    chunk_mms(po, mq, 0, True, False)
                for mq in range(4):
                    chunk_mms(pos[mq], mq, 1, False, True)
                    evac(pos[mq], mq)

        rr = rope.rearrange("r p s -> p r s")
        xn_t, rsb_t = xn0, rsb0
        for n in range(NQT):
            proj(n, xn_t, rsb_t)
            if n + 1 < NQT:
                ns2 = slice((n + 1) * 512, (n + 2) * 512)
                xn_t = sb.tile([P, KO, 512], dt.bfloat16, tag="xn", bufs=1,
                               name="xn_n")
                nc.sync.dma_start(xn_t[:], xTr[:, :, ns2])
                rsb_t = sb.tile([P, 2, 512], dt.float32, tag="rsb", bufs=2,
                                name="rsb_n")
                nc.sync.dma_start(rsb_t[:], rr[:, :, ns2])
            attention(n)
            if n >= 2:
                out_proj(n - 2)
        out_proj(NQT - 2)
        out_proj(NQT - 1, split=True)

    nc.compile()
    return nc


def _host_prep(x, attention_mask, frequency_cis, Wqkv, Wout):
    """Build the 8 per-core input maps (numpy only)."""
    x = np.asarray(x, dtype=np.float32)
    fc = np.asarray(frequency_cis, dtype=np.float32)
    Wqkv = np.asarray(Wqkv, dtype=np.float32)
    Wout = np.asarray(Wout, dtype=np.float32)

    # rotate-half permutation of the head dim: new row p<64 <- old 2p,
    # p>=64 <- old 2(p-64)+1
    perm = np.concatenate([np.arange(0, HD, 2), np.arange(1, HD, 2)])
    # rope coefficients in permuted layout: [A;B] each [HD, S]
    ropeA = np.concatenate([fc[:, :, 0, 0].T, fc[:, :, 1, 1].T], axis=0)
    ropeBsw = np.concatenate([fc[:, :, 1, 0].T, fc[:, :, 0, 1].T], axis=0)
    rope = np.stack([ropeA, ropeBsw]).astype(np.float32)  # [2, HD, S]

    # strict upper triangle masked: key i > query c
    mtri = np.where(np.arange(128)[:, None] > np.arange(128)[None, :],
                    np.float32(NEG), np.float32(0.0)).astype(np.float32)

    xT = [np.ascontiguousarray(x[b].T).astype(BF16) for b in range(B)]
    woutT_f = Wout.T.astype(np.float32)                  # [H(in), H(out)]
    wout_slices = [np.ascontiguousarray(
        woutT_f[:, g * 512:(g + 1) * 512]).astype(BF16) for g in range(4)]

    in_maps = []
    for c in range(NCORES):
        b, g = divmod(c, 4)
        qk_rows = []
        v_rows = []
        for j in range(NHL):
            hh = (g * NHL + j) * HD
            qk_rows.append(Wqkv[0 * H + hh:0 * H + hh + HD][perm] * SCALE)
            qk_rows.append(Wqkv[1 * H + hh:1 * H + hh + HD][perm])
            v_rows.append(Wqkv[2 * H + hh:2 * H + hh + HD])
        # block-major: [8 blocks, H, 128], block i = rows of (head i//2,
        # q if i%2==0 else k)
        wqk = np.stack([r.T for r in qk_rows])           # [8, H, 128]
        wv = np.concatenate(v_rows, axis=0)              # [512, H]
        in_maps.append({
            "xT": xT[b],
            "wqkT": np.ascontiguousarray(wqk).astype(BF16),
            "wvT": np.ascontiguousarray(wv.T).astype(BF16),
            "rope": rope,
            "mtri": mtri,
            "woutT": wout_slices[g],
        })
    return in_maps


def _install_ntff_hook():
    """The image's antenv lacks axon_hooks; shim it so trace=True works."""
    import sys
    import types
    import ctypes
    import contextlib
    if "antenv.axon_hooks" in sys.modules:
        return
    mod = types.ModuleType("antenv.axon_hooks")
    _reg = {"hook": None}
    mod.set_axon_ntff_profile_hook = lambda h: _reg.__setitem__("hook", h)
    mod.get_axon_ntff_profile_hook = lambda: _reg["hook"]
    sys.modules["antenv.axon_hooks"] = mod

    so_path = "/opt/axon/libaxon_pjrt.so"
    try:
        lib = ctypes.CDLL(so_path)
        if not hasattr(lib, "axon_start_nrt_profile"):
            return
        lib.axon_start_nrt_profile.argtypes = [
            ctypes.POINTER(ctypes.c_int64), ctypes.c_size_t]
        lib.axon_start_nrt_profile.restype = ctypes.c_int64
        lib.axon_stop_nrt_profile.argtypes = [ctypes.c_char_p]
        lib.axon_stop_nrt_profile.restype = ctypes.c_int64

        @contextlib.contextmanager
        def _hook(output_dir, device_ids):
            import jax
            jax.devices()
            if device_ids:
                ids = (ctypes.c_int64 * len(device_ids))(*device_ids)
                rc = lib.axon_start_nrt_profile(ids, len(device_ids))
            else:
                rc = lib.axon_start_nrt_profile(None, 0)
            if rc != 0:
                raise RuntimeError(f"axon_start_nrt_profile rc={rc}")
            try:
                yield
            finally:
                n = lib.axon_stop_nrt_profile(str(output_dir).encode())
                print(f"profile: {n} file(s) written to {output_dir}")

        mod.set_axon_ntff_profile_hook(_hook)
    except OSError:
        pass


def _run(in_maps, trace=False):
    if trace:
        _install_ntff_hook()
    from concourse.bass_utils import run_bass_kernel_spmd
    if "nc" not in _cache:
        _cache["nc"] = _build()
    return run_bass_kernel_spmd(_cache["nc"], in_maps,
                                list(range(NCORES)), trace=trace)


def _assemble(r):
    out = np.empty((B, S, H), dtype=np.float32)
    for c in range(NCORES):
        b, g = divmod(c, 4)
        out[b, :, g * 512:(g + 1) * 512] = r.results[c]["out"]
    return out


def kernel(x, attention_mask, frequency_cis, Wqkv, Wout):
    in_maps = _host_prep(x, attention_mask, frequency_cis, Wqkv, Wout)
    r = _run(in_maps)
    return _assemble(r)


def kernel_traced(x, attention_mask, frequency_cis, Wqkv, Wout):
    """Like kernel() but also returns (out, exec_time_ns)."""
    in_maps = _host_prep(x, attention_mask, frequency_cis, Wqkv, Wout)
    r = _run(in_maps, trace=True)
    return _assemble(r), getattr(r, "exec_time_ns", None)
